# revision 4
# baseline (speedup 1.0000x reference)
"""Trainium2 Bass kernel for nn_CC_Decoder (hypernetwork-decoded per-pixel MLP).

Strategy (8 NeuronCores, data-parallel over batch: one sample per core):

Reference computation per sample:
  W_raw = conv1x1(x)                         # [1028, 256] channel matmul
  Wf    = W_raw @ wfine^T + wfine_b          # [1028, 256]
  layer j weights wj = Wf[257j : 257j+256], bias bj = Wf[257j+256]
  out = PE(coords)  -> 4 x (out @ wj + bj -> PReLU) -> last1 -> SiLU

v2: the four generated-layer GEMMs run as fp8e4 DoubleRow matmuls: weights
and activations are stored as [128 part, 2 k-chunk, free] fp8 tiles, so one
matmul contracts all K=256 at 2x bf16 throughput. Power-of-2 scales keep
fp8 operands in range (PReLU commutes with positive scales; the cumulative
scale is undone by the SiLU activation's scale port). The weight-generation
phases (conv, wfine), the biases, and the last1 layer stay bf16/fp32: the
network output is bias-dominated, so fp8 noise on the matmul paths is
strongly attenuated while the bias path keeps full precision.

The positional-encoding input x2 is an outer sum over (y, x):
x2[(y,x), :] = [u(y)(128) | v(x)(128)] with u = v = T columns. Layer 0's
DoubleRow rhs is a host fp8 table [128, 2, 2048] per superpair: slot 0 the
y-columns (broadcast along x), slot 1 the x-columns (tiled) - the 16 MB x2
tensor never materializes.

Per-layer PReLU+bias is split statically across three engines: ACT does
prelu(scale*psum + bias) in one op; the rest run as a DVE fused op
t = (psum + b')*(alpha*u) followed by max(t/alpha, t) on the otherwise-idle
GPSIMD (or DVE) via scalar_tensor_tensor. The last1 (256->3) matmuls stack
4 pixel-tiles into one PSUM bank at 32-aligned partition offsets via
tile_position col-groups, amortizing SiLU to one instruction per 2048 px.
"""
import numpy as np
import ml_dtypes

bf16 = ml_dtypes.bfloat16
f8 = ml_dtypes.float8_e4m3fn

IMG = 128
NPX = IMG * IMG          # 16384 pixels
NF = 256                 # feature width
C1 = 1024                # conv in-channels
WD = 1028                # conv out-channels (= 4*257)
L = 4                    # generated layers
C2 = 3                   # output channels
TP = 512                 # pixel tile
NT = NPX // TP           # 32 tiles
NSP = NT // 4            # 8 superpairs (2048 px each)
M_ = 64
SIGMA = 10.0

# power-of-2 fp8 scale plan: stored_act_j = G[j] * act_j, stored_w_j = AW * w_j,
# stored layer-0 input tables = GIN * x2. U[j] = G[j]/(AW*Gprev[j]) is the
# psum prescale that recovers G[j]*(w@act + b) before PReLU.
GIN = 8.0
AW = 4096.0
G = [1024.0, 8192.0, 8192.0, 8192.0]
GPREV = [GIN, G[0], G[1], G[2]]
U = [G[j] / (AW * GPREV[j]) for j in range(L)]

# static engine assignment for the per-(layer, chunk, h) PReLU ops:
# 'A' = one ACT op; 'D' = DVE fused step1 (tensor_scalar, 2 scalar ops) +
# DVE step2 (scalar_tensor_tensor max). GPSIMD can't help: this walrus
# build rejects tensor_tensor / scalar_tensor_tensor on Pool, and PReLU is
# not expressible as a single-input scalar-op chain.
# Layer 3 stays on ACT (bf16 output for last1).
PATH = {
    (0, 0, 0): 'D', (0, 0, 1): 'D', (0, 1, 0): 'D', (0, 1, 1): 'D',
    (1, 0, 0): 'D', (1, 0, 1): 'D',
}

_last_results = None     # stash for test.py introspection


def _host_tables():
    v0, v1 = -0.99999, 1.0
    r = (v1 - v0) / (2 * IMG)
    seq = v0 + r + 2 * r * np.arange(IMG, dtype=np.float64)
    j = np.arange(M_, dtype=np.float64)
    coeffs = 2.0 * np.pi * (SIGMA ** (j / M_))
    vp = coeffs[:, None] * seq[None, :]          # [64, 128]
    T = np.concatenate([np.cos(vp), np.sin(vp)], axis=0)  # [128, 128]
    return T.astype(np.float32)


def _build_program(alpha: float):
    import concourse.bass as bass
    import concourse.mybir as mybir
    import concourse.tile as tile
    import bir_patch_embedded  # installed below via sys.modules
    bir_patch_embedded.install()

    fp = mybir.dt.float32
    bf = mybir.dt.bfloat16
    f8d = mybir.dt.float8e4
    PRELU = mybir.ActivationFunctionType.Prelu
    SILU = mybir.ActivationFunctionType.Silu
    ADD = mybir.AluOpType.add
    MULT = mybir.AluOpType.mult
    MAX = mybir.AluOpType.max
    DR = mybir.MatmulPerfMode.DoubleRow

    # DVE/GPSIMD prelu path needs 0 < alpha <= 1 (max identity + finite 1/a)
    dve_ok = 1e-3 <= alpha <= 1.0

    nc = bass.Bass()
    xb_d = nc.declare_dram_parameter("xb", [128, 8, NF], bf, isOutput=False)
    cwT_d = nc.declare_dram_parameter("cwT", [128, 8, WD], bf, isOutput=False)
    cb_d = nc.declare_dram_parameter("cb", [1, WD], bf, isOutput=False)
    wfT_d = nc.declare_dram_parameter("wfT", [128, 2, NF], bf, isOutput=False)
    wfb_d = nc.declare_dram_parameter("wfb", [1, NF], bf, isOutput=False)
    lwT_d = nc.declare_dram_parameter("lwT", [128, 2, C2], bf, isOutput=False)
    lbrep_d = nc.declare_dram_parameter("lbrep", [128, 1], fp, isOutput=False)
    Ty8_d = nc.declare_dram_parameter("Ty8", [128, NSP, 2, 4 * TP], f8d,
                                      isOutput=False)
    out_d = nc.declare_dram_parameter("out", [C2, NPX], fp, isOutput=True)
    out_r = out_d.rearrange("c (t x) -> c t x", x=TP)

    with tile.TileContext(nc) as tc:
        with (
            tc.tile_pool(name="wpool", bufs=1) as wp,
            tc.tile_pool(name="actp", bufs=4) as ap,
            tc.tile_pool(name="dvet", bufs=5) as dp,
            tc.tile_pool(name="outp", bufs=2) as op,
            tc.tile_pool(name="psmain", bufs=3, space="PSUM") as psm,
            tc.tile_pool(name="pslast", bufs=2, space="PSUM") as psl,
        ):
            # ---- persistent weights / tables ----
            xb = wp.tile([128, 8, NF], bf)
            cwT = wp.tile([128, 8, WD], bf)
            cb = wp.tile([1, WD], bf)
            wfT = wp.tile([128, 2, NF], bf)
            wfb = wp.tile([1, NF], bf)
            lwT = wp.tile([128, 2, C2], bf)
            lbrep = wp.tile([128, 1], fp)
            ones = wp.tile([1, 128], bf)
            Wt = wp.tile([128, 2, WD], bf)           # conv out, transposed (W^T)
            wj8 = [wp.tile([128, 2, NF], f8d, tag=f"wj{j}", name=f"wj{j}")
                   for j in range(L)]
            bA = [wp.tile([128, 2], fp, tag=f"bA{j}", name=f"bA{j}")
                  for j in range(L)]
            bD = [wp.tile([128, 2], fp, tag=f"bD{j}", name=f"bD{j}")
                  for j in range(L)]

            nc.sync.dma_start(xb[:], xb_d[:])
            for q in range(8):
                nc.sync.dma_start(cwT[:, q, :], cwT_d[:, q, :])
            nc.sync.dma_start(cb[:], cb_d[:])
            nc.sync.dma_start(wfT[:], wfT_d[:])
            nc.sync.dma_start(wfb[:], wfb_d[:])
            nc.sync.dma_start(lwT[:], lwT_d[:])
            nc.sync.dma_start(lbrep[:], lbrep_d[:])
            nc.vector.memset(ones[:], 1.0)

            # HAM warmup: junk matmuls on an uninitialized tile keep the PE
            # busy during the input DMA so phase A starts at full clock.
            junk = wp.tile([128, 512], bf)
            nc.vector.memset(junk[:], 0.5)
            jps = psm.tile([128, 2, TP], fp, tag="psmm", name="warm")
            jpsf = jps.rearrange("p a b -> p (a b)")
            for i in range(10):
                nc.tensor.matmul(jpsf[:, 0:512], junk[:, 0:128], junk[:],
                                 start=(i == 0), stop=(i == 9))

            # ---- phase A: conv (1x1) -> W^T [hw=256 on 2 chunks, 1028 free] ----
            # Slice (0,512) first: it alone feeds layer-0 weights (rows
            # 0..257), so phase B j=0 + the main loop's first superpair can
            # start while the remaining conv columns are still computing.
            for off, sz in ((0, 512),):
                for m in range(2):
                    ps = psm.tile([128, 2, TP], fp, tag="psmm", name="psA0")
                    psf = ps.rearrange("p a b -> p (a b)")
                    for q in range(8):
                        nc.tensor.matmul(
                            psf[:, :sz], xb[:, q, 128 * m:128 * (m + 1)],
                            cwT[:, q, off:off + sz],
                            start=(q == 0), stop=False)
                    nc.tensor.matmul(
                        psf[:, :sz], ones[:, 0:128], cb[:, off:off + sz],
                        start=False, stop=True)
                    nc.vector.tensor_copy(Wt[:, m, off:off + sz], psf[:, :sz])

            def emit_phaseB(j):
                r0 = 257 * j
                for m in range(2):
                    ps = psm.tile([128, 2, TP], fp, tag="psmm",
                                  name=f"psB{j}{m}")
                    psf = ps.rearrange("p a b -> p (a b)")[:, :NF]
                    for k in range(2):
                        nc.tensor.matmul(
                            psf[:], Wt[:, k, r0 + 128 * m:r0 + 128 * (m + 1)],
                            wfT[:, k, :], start=(k == 0), stop=False)
                    nc.tensor.matmul(psf[:], ones[:, 0:128], wfb[:],
                                     start=False, stop=True)
                    nc.vector.tensor_scalar(wj8[j][:, m, :], psf[:],
                                            AW, None, MULT)
                for c in range(2):
                    psb = psm.tile([128, 2, TP], fp, tag="psmm",
                                   name=f"psBb{j}{c}")
                    psbf = psb.rearrange("p a b -> p (a b)")[:, :1]
                    for k in range(2):
                        nc.tensor.matmul(
                            psbf[:], wfT[:, k, 128 * c:128 * (c + 1)],
                            Wt[:, k, r0 + 256:r0 + 257],
                            start=(k == 0), stop=False)
                    nc.tensor.matmul(psbf[:], wfb[:, 128 * c:128 * (c + 1)],
                                     ones[:, 0:1], start=False, stop=True)
                    nc.vector.tensor_scalar(bA[j][:, c:c + 1], psbf[:],
                                            G[j], None, MULT)
                    nc.vector.tensor_scalar(bD[j][:, c:c + 1], psbf[:],
                                            AW * GPREV[j], None, MULT)

            # phase B j=0 right after the hoisted conv slice so the first
            # superpair's layer 0 can start early
            emit_phaseB(0)

            for m in range(2):
                for off, sz in ((512, 512), (1024, 4)):
                    ps = psm.tile([128, 2, TP], fp, tag="psmm", name="psA")
                    psf = ps.rearrange("p a b -> p (a b)")
                    for q in range(8):
                        nc.tensor.matmul(
                            psf[:, :sz], xb[:, q, 128 * m:128 * (m + 1)],
                            cwT[:, q, off:off + sz],
                            start=(q == 0), stop=False)
                    nc.tensor.matmul(
                        psf[:, :sz], ones[:, 0:128], cb[:, off:off + sz],
                        start=False, stop=True)
                    nc.vector.tensor_copy(Wt[:, m, off:off + sz], psf[:, :sz])

            for j in [1, 2, 3]:
                emit_phaseB(j)

            # ---- main loop: superpairs of 2048 px, DoubleRow fp8 layers ----
            def emit_layer(sp, j, prev, Ty_sb):
                odt = bf if j == L - 1 else f8d
                actj = [ap.tile([128, 2, 2 * TP], odt, tag=f"act{j}{h}",
                                name=f"act{j}{h}_{sp}") for h in range(2)]
                for c in range(2):
                    ps = [psm.tile([128, 2, TP], fp, tag="psmm",
                                   name=f"ps{j}{c}{h}_{sp}") for h in range(2)]
                    for h in range(2):
                        for s_ in range(2):
                            if j == 0:
                                rhs = Ty_sb[:, 0:2,
                                            1024 * h + TP * s_:
                                            1024 * h + TP * (s_ + 1)]
                            else:
                                rhs = prev[h][:, 0:2, TP * s_:TP * (s_ + 1)]
                            nc.tensor.matmul(
                                ps[h][:, s_, :],
                                wj8[j][:, 0:2, 128 * c:128 * (c + 1)], rhs,
                                start=True, stop=True, perf_mode=DR)
                    for h in range(2):
                        psf = ps[h].rearrange("p a b -> p (a b)")
                        path = PATH.get((j, c, h), 'A') if dve_ok else 'A'
                        if path == 'A':
                            nc.scalar.activation(
                                actj[h][:, c, :], psf, PRELU,
                                bias=bA[j][:, c:c + 1], scale=U[j],
                                alpha=alpha)
                        else:
                            t = dp.tile([128, 2 * TP], bf, tag="dvet",
                                        name=f"t{j}{c}{h}_{sp}")
                            nc.vector.tensor_scalar(
                                t[:], psf, bD[j][:, c:c + 1],
                                alpha * U[j], ADD, MULT)
                            nc.vector.scalar_tensor_tensor(
                                actj[h][:, c, :], t[:], 1.0 / alpha, t[:],
                                MULT, MAX)
                return actj

            def emit_last(sp, act3):
                accL = psl.tile([128, TP], fp, tag="pslastb", name=f"accL{sp}")
                for k in range(2):
                    for q in range(4):
                        h, s_ = q // 2, q % 2
                        nc.tensor.matmul(
                            accL[32 * q:32 * q + C2, :], lwT[:, k, :],
                            act3[h][:, k, TP * s_:TP * (s_ + 1)],
                            start=(k == 0), stop=(k == 1),
                            tile_position=(0, 32 * q))
                souf = op.tile([128, TP], fp, tag="souf", name=f"souf{sp}")
                nc.scalar.activation(souf[0:99, :], accL[0:99, :],
                                     SILU, bias=lbrep[0:99, 0:1],
                                     scale=1.0 / G[L - 1])
                for c in range(C2):
                    nc.sync.dma_start(out_r[c, 4 * sp:4 * sp + 4, :],
                                      souf[c:c + 97:32, :])

            # 4-deep layer-skewed software pipeline: at step s emit
            # L3(s-3), L2(s-2), L1(s-1), L0(s) - oldest stream first so the
            # critical path gets scheduler priority, newest fills gaps.
            acts = {}
            for step in range(NSP + 3):
                for j in (3, 2, 1, 0):
                    sp = step - j
                    if not (0 <= sp < NSP):
                        continue
                    if j == 0:
                        Ty_sb = ap.tile([128, 2, 4 * TP], f8d, tag="tysb",
                                        name=f"ty{sp}")
                        nc.sync.dma_start(Ty_sb[:], Ty8_d[:, sp])
                        acts[(sp, 0)] = emit_layer(sp, 0, None, Ty_sb)
                    else:
                        acts[(sp, j)] = emit_layer(
                            sp, j, acts.pop((sp, j - 1)), None)
                    if j == 3:
                        emit_last(sp, acts.pop((sp, 3)))
                if 1 <= step <= 3:
                    # keep HAM warm through pipeline-fill stalls
                    for i in range(2):
                        nc.tensor.matmul(jpsf[:, 0:512], junk[:, 0:128],
                                         junk[:], start=(i == 0),
                                         stop=(i == 1))
    return nc


def kernel(x, conv_w, conv_b, wfine_w, wfine_b, last1_w, last1_b, prelu_a,
           **_ignored):
    global _last_results
    from concourse.bass_utils import run_bass_kernel_spmd

    x = np.asarray(x)
    B = x.shape[0]
    assert x.shape == (B, C1, 16, 16) and B == 8, x.shape

    conv_w = np.asarray(conv_w, np.float32)      # [1028, 1024]
    conv_b = np.asarray(conv_b, np.float32)      # [1028]
    wfine_w = np.asarray(wfine_w, np.float32)    # [256, 256]
    wfine_b = np.asarray(wfine_b, np.float32)    # [256]
    last1_w = np.asarray(last1_w, np.float32)    # [3, 256]
    last1_b = np.asarray(last1_b, np.float32)    # [3]
    alpha = float(np.asarray(prelu_a).reshape(-1)[0])

    # host-side shared operands (bf16)
    cwT = np.ascontiguousarray(
        conv_w.T.reshape(8, 128, WD).transpose(1, 0, 2)).astype(bf16)
    cb = conv_b.reshape(1, WD).astype(bf16)
    wfT = np.ascontiguousarray(
        wfine_w.T.reshape(2, 128, NF).transpose(1, 0, 2)).astype(bf16)
    wfb = wfine_b.reshape(1, NF).astype(bf16)
    lwT = np.ascontiguousarray(
        last1_w.T.reshape(2, 128, C2).transpose(1, 0, 2)).astype(bf16)
    lbrep = np.zeros((128, 1), np.float32)
    for g in range(4):
        lbrep[32 * g:32 * g + C2, 0] = last1_b
    # layer-0 DoubleRow rhs tables, pre-scaled by GIN: slot 0 = y-part
    # columns (broadcast along x), slot 1 = x-part columns (tiled 16x)
    T8 = GIN * _host_tables()
    Ty8 = np.empty((128, NSP, 2, 4 * TP), np.float32)
    xs = np.tile(np.arange(IMG), 16)
    for sp in range(NSP):
        ys = 16 * sp + np.repeat(np.arange(16), IMG)
        Ty8[:, sp, 0, :] = T8[:, ys]
        Ty8[:, sp, 1, :] = T8[:, xs]
    Ty8 = Ty8.astype(f8)

    nc = _build_program(alpha)

    in_maps = []
    for b in range(B):
        xb = np.ascontiguousarray(
            x[b].reshape(8, 128, NF).transpose(1, 0, 2)).astype(bf16)
        in_maps.append({"xb": xb, "cwT": cwT, "cb": cb, "wfT": wfT,
                        "wfb": wfb, "lwT": lwT, "lbrep": lbrep,
                        "Ty8": Ty8})

    res = run_bass_kernel_spmd(nc, in_maps, list(range(8)))
    _last_results = res
    out = np.stack([res.results[b]["out"].reshape(C2, IMG, IMG)
                    for b in range(B)])
    return out.astype(np.float32)


# ---------------------------------------------------------------------------
# Embedded walrus workaround (kernel.py must be self-contained): this walrus
# build accepts at most ONE sync wait per instruction; Tile attaches several.
# Split them into preceding single-wait NoOps at the BIR-JSON level, and make
# the TileContext tail drain emit one single-wait drain per logical proc.
# ---------------------------------------------------------------------------
import sys as _sys
import types as _types

_patch_mod = _types.ModuleType("bir_patch_embedded")
_patch_src = r'''
import json

def install():
    import concourse.bass_utils as _bu
    import concourse.bass2jax as _b2j
    import concourse.tile as _tile
    from concourse.vector_clock import ScopedClock, VectorClock

    if getattr(_bu, "_wait_legalizer_installed", False):
        return
    _bu._wait_legalizer_installed = True
    _orig_compile = _bu.compile_bir_kernel

    def _merge_ldweights(m):
        """Re-merge tile-legalize's split Ldweights into self-loading
        Matmults so walrus codegen can apply FWL / ldw dedupe."""
        for fn in m.get("functions", []):
            for bb in fn.get("blocks", []):
                instrs = bb.get("instructions", [])
                out = []
                i = 0
                while i < len(instrs):
                    ins = instrs[i]
                    if ins.get("opcode") == "Ldweights":
                        wap = json.dumps(ins["ins"][0], sort_keys=True)
                        # find the next Matmult on this engine using these
                        # weights (stationary operand = ins[1])
                        tgt = None
                        for k in range(i + 1, min(i + 8, len(instrs))):
                            nxt = instrs[k]
                            if nxt.get("engine") != ins.get("engine"):
                                continue
                            if nxt.get("opcode") == "Matmult" and json.dumps(
                                    nxt["ins"][1], sort_keys=True) == wap:
                                tgt = nxt
                            break
                        if tgt is not None:
                            tgt["ldweights"] = True
                            si, ti = ins.get("sync_info") or {}, tgt.setdefault(
                                "sync_info", {"on_wait": [], "on_update": []})
                            ti.setdefault("on_wait", []).extend(
                                si.get("on_wait") or [])
                            ti.setdefault("on_update", []).extend(
                                si.get("on_update") or [])
                            i += 1
                            continue
                    out.append(ins)
                    i += 1
                bb["instructions"] = out
        return m

    def _legalize_waits(bir_json):
        m = json.loads(bir_json)
        m = _merge_ldweights(m)
        cnt = 0
        changed = True
        for fn in m.get("functions", []):
            for bb in fn.get("blocks", []):
                new_instrs = []
                for ins in bb.get("instructions", []):
                    si = ins.get("sync_info")
                    ow = (si or {}).get("on_wait") or []
                    if len(ow) > 1:
                        changed = True
                        for w in ow[:-1]:
                            cnt += 1
                            new_instrs.append({
                                "engine": ins["engine"],
                                "ins": [], "outs": [],
                                "name": "WSPLIT-%d" % cnt,
                                "opcode": "NoOp",
                                "sync_info": {"on_update": [], "on_wait": [w]},
                                "debug": ins.get("debug", 0),
                            })
                        si["on_wait"] = [ow[-1]]
                    new_instrs.append(ins)
                bb["instructions"] = new_instrs
        if not changed:
            return bir_json
        return json.dumps(m).encode()

    def _compile_legalized(bir_json, tmpdir, neff_name="file.neff"):
        return _orig_compile(_legalize_waits(bir_json), tmpdir, neff_name)

    _bu.compile_bir_kernel = _compile_legalized
    _b2j.compile_bir_kernel = _compile_legalized

    import os
    if os.environ.get("BASS_LDW_OPT", "1") != "0":
        _orig_verify = _bu.bir_verify_and_optimise

        def _verify_ldwopt(tmpdir, inp="bir.json", outp="file.neff", arch=None,
                           *, dve_root=None):
            saved = _bu.run_command

            def run_cmd(cmd, **kw):
                cmd = [c.replace("--enable-ldw-opt=false",
                                 "--enable-ldw-opt=true")
                       if isinstance(c, str) else c for c in cmd]
                return saved(cmd, **kw)
            _bu.run_command = run_cmd
            try:
                return _orig_verify(tmpdir, inp, outp, arch, dve_root=dve_root)
            finally:
                _bu.run_command = saved
        _bu.bir_verify_and_optimise = _verify_ldwopt

    def _drain_and_barrier_split(self, tick_clock, wait_clock):
        nc = self.nc
        vclock = tick_clock.global_clock
        n = len(vclock)
        for p in range(n):
            t = vclock[p]
            if t <= 0:
                continue
            v = VectorClock([0] * n)
            v.require_at_least(p, t)
            d = nc.sync.drain()
            wait_clock.add_sem_waits(d.ins, ScopedClock({None: v}))
        nc.all_engine_barrier()
        popped = nc._tile_sem_poison_stack.pop()
        assert popped is self._sem_poison
        nc.clear_and_free_semaphores(list(self.sems.allocated().values()))
        nc.all_engine_barrier()
    _tile.TileContext._drain_and_barrier = _drain_and_barrier_split
'''
exec(_patch_src, _patch_mod.__dict__)
_sys.modules["bir_patch_embedded"] = _patch_mod


# revision 21
# speedup vs baseline: 1.0816x; 1.0816x over previous
"""Trainium2 Bass kernel for nn_CC_Decoder (hypernetwork-decoded per-pixel MLP).

Strategy (8 NeuronCores, data-parallel over batch: one sample per core):

Reference computation per sample:
  W_raw = conv1x1(x)                         # [1028, 256] channel matmul
  Wf    = W_raw @ wfine^T + wfine_b          # [1028, 256]
  layer j weights wj = Wf[257j : 257j+256], bias bj = Wf[257j+256]
  out = PE(coords)  -> 4 x (out @ wj + bj -> PReLU) -> last1 -> SiLU

v2: the four generated-layer GEMMs run as fp8e4 DoubleRow matmuls: weights
and activations are stored as [128 part, 2 k-chunk, free] fp8 tiles, so one
matmul contracts all K=256 at 2x bf16 throughput. Power-of-2 scales keep
fp8 operands in range (PReLU commutes with positive scales; the cumulative
scale is undone by the SiLU activation's scale port). The weight-generation
phases (conv, wfine), the biases, and the last1 layer stay bf16/fp32: the
network output is bias-dominated, so fp8 noise on the matmul paths is
strongly attenuated while the bias path keeps full precision.

The positional-encoding input x2 is an outer sum over (y, x):
x2[(y,x), :] = [u(y)(128) | v(x)(128)] with u = v = T columns. Layer 0's
DoubleRow rhs is a host fp8 table [128, 2, 2048] per superpair: slot 0 the
y-columns (broadcast along x), slot 1 the x-columns (tiled) - the 16 MB x2
tensor never materializes.

Per-layer PReLU+bias is split statically across three engines: ACT does
prelu(scale*psum + bias) in one op; the rest run as a DVE fused op
t = (psum + b')*(alpha*u) followed by max(t/alpha, t) on the otherwise-idle
GPSIMD (or DVE) via scalar_tensor_tensor. The last1 (256->3) matmuls stack
4 pixel-tiles into one PSUM bank at 32-aligned partition offsets via
tile_position col-groups, amortizing SiLU to one instruction per 2048 px.
"""
import numpy as np
import ml_dtypes

bf16 = ml_dtypes.bfloat16
f8 = ml_dtypes.float8_e4m3fn

IMG = 128
NPX = IMG * IMG          # 16384 pixels
NF = 256                 # feature width
C1 = 1024                # conv in-channels
WD = 1028                # conv out-channels (= 4*257)
L = 4                    # generated layers
C2 = 3                   # output channels
TP = 512                 # pixel tile
NT = NPX // TP           # 32 tiles
NSP = NT // 4            # 8 superpairs (2048 px each)
M_ = 64
SIGMA = 10.0

# power-of-2 fp8 scale plan: stored_act_j = G[j] * act_j, stored_w_j = AW * w_j,
# stored layer-0 input tables = GIN * x2. U[j] = G[j]/(AW*Gprev[j]) is the
# psum prescale that recovers G[j]*(w@act + b) before PReLU.
GIN = 8.0
AW = 4096.0
G = [1024.0, 8192.0, 8192.0, 8192.0]
GPREV = [GIN, G[0], G[1], G[2]]
U = [G[j] / (AW * GPREV[j]) for j in range(L)]

# static engine assignment for the per-(layer, chunk, h) PReLU ops:
# 'A' = one ACT op (prelu with bias+scale ports, reads PSUM directly);
# 'D' = DVE pair: fused tensor_scalar t=(psum+b')*(alpha*u), then
# scalar_tensor_tensor out=max(t/alpha, t). The DVE pairs go on layer 3
# because its output is bf16 (for last1): a 2-byte output keeps the DVE
# 2x mode on step 2 (~650 ns vs ~1215 with an fp8 output). GPSIMD can't
# help (no PSUM port; this walrus rejects 2-input ops on Pool) and custom
# DVE ops fail walrus codegen entirely.
PATH = {
    (3, 0, 0): 'D', (3, 0, 1): 'D', (3, 1, 0): 'D', (3, 1, 1): 'D',
    (0, 0, 0): 'D',
}

_last_results = None     # stash for test.py introspection


def _host_tables():
    v0, v1 = -0.99999, 1.0
    r = (v1 - v0) / (2 * IMG)
    seq = v0 + r + 2 * r * np.arange(IMG, dtype=np.float64)
    j = np.arange(M_, dtype=np.float64)
    coeffs = 2.0 * np.pi * (SIGMA ** (j / M_))
    vp = coeffs[:, None] * seq[None, :]          # [64, 128]
    T = np.concatenate([np.cos(vp), np.sin(vp)], axis=0)  # [128, 128]
    return T.astype(np.float32)


def _build_program(alpha: float):
    import concourse.bass as bass
    import concourse.mybir as mybir
    import concourse.tile as tile
    import bir_patch_embedded  # installed below via sys.modules
    bir_patch_embedded.install()

    fp = mybir.dt.float32
    bf = mybir.dt.bfloat16
    f8d = mybir.dt.float8e4
    PRELU = mybir.ActivationFunctionType.Prelu
    SILU = mybir.ActivationFunctionType.Silu
    ADD = mybir.AluOpType.add
    MULT = mybir.AluOpType.mult
    MAX = mybir.AluOpType.max
    DR = mybir.MatmulPerfMode.DoubleRow

    # DVE prelu path needs 0 < alpha <= 1 (max identity + finite 1/alpha)
    dve_ok = 1e-3 <= alpha <= 1.0
    bD_layers = sorted({j for (j, _, _) in PATH}) if dve_ok else []

    nc = bass.Bass()
    xb_d = nc.declare_dram_parameter("xb", [128, 8, NF], bf, isOutput=False)
    cwT_d = nc.declare_dram_parameter("cwT", [128, 8, WD], bf, isOutput=False)
    cb_d = nc.declare_dram_parameter("cb", [1, WD], bf, isOutput=False)
    wfT_d = nc.declare_dram_parameter("wfT", [128, 2, NF], bf, isOutput=False)
    wfb_d = nc.declare_dram_parameter("wfb", [1, NF], bf, isOutput=False)
    lwT_d = nc.declare_dram_parameter("lwT", [128, 2, C2], bf, isOutput=False)
    lbrep_d = nc.declare_dram_parameter("lbrep", [128, 1], fp, isOutput=False)
    Ty8_d = nc.declare_dram_parameter("Ty8", [128, NSP, 2, 4 * TP], f8d,
                                      isOutput=False)
    out_d = nc.declare_dram_parameter("out", [C2, NPX], fp, isOutput=True)
    out_r = out_d.rearrange("c (t x) -> c t x", x=TP)

    with tile.TileContext(nc) as tc:
        with (
            tc.tile_pool(name="wpool", bufs=1) as wp,
            tc.tile_pool(name="actp", bufs=4) as ap,
            tc.tile_pool(name="dvet", bufs=5) as dp,
            tc.tile_pool(name="outp", bufs=2) as op,
            tc.tile_pool(name="psmain", bufs=3, space="PSUM") as psm,
            tc.tile_pool(name="pslast", bufs=1, space="PSUM") as psl,
            tc.tile_pool(name="pswarm", bufs=1, space="PSUM") as psw,
        ):
            # ---- persistent weights / tables ----
            xb = wp.tile([128, 8, NF], bf)
            cwT = wp.tile([128, 8, WD], bf)
            cb = wp.tile([1, WD], bf)
            wfT = wp.tile([128, 2, NF], bf)
            wfb = wp.tile([1, NF], bf)
            lwT = wp.tile([128, 2, C2], bf)
            lbrep = wp.tile([128, 1], fp)
            ones = wp.tile([1, 128], bf)
            Wt = wp.tile([128, 2, WD], bf)           # conv out, transposed (W^T)
            wj8 = [wp.tile([128, 2, NF], f8d, tag=f"wj{j}", name=f"wj{j}")
                   for j in range(L)]
            bA = [wp.tile([128, 2], fp, tag=f"bA{j}", name=f"bA{j}")
                  for j in range(L)]
            bD = {j: wp.tile([128, 2], fp, tag=f"bD{j}", name=f"bD{j}")
                  for j in bD_layers}

            nc.sync.dma_start(xb[:], xb_d[:])
            for q in range(8):
                nc.sync.dma_start(cwT[:, q, :], cwT_d[:, q, :])
            nc.sync.dma_start(cb[:], cb_d[:])
            nc.sync.dma_start(wfT[:], wfT_d[:])
            nc.sync.dma_start(wfb[:], wfb_d[:])
            nc.sync.dma_start(lwT[:], lwT_d[:])
            nc.sync.dma_start(lbrep[:], lbrep_d[:])
            nc.vector.memset(ones[:], 1.0)

            # HAM warmup + fillers: junk matmuls on a PRIVATE psum bank (own
            # WAW chain, no ring deps) execute exactly when the PE would
            # otherwise idle on ring waits, keeping the clock at 2.4 GHz.
            junk = wp.tile([128, 512], bf)
            nc.vector.memset(junk[:], 0.5)
            jps = psw.tile([128, 512], fp, tag="pswarmb", name="warm")

            def emit_filler(n=1, sz=256):
                for _ in range(n):
                    nc.tensor.matmul(jps[:, 0:sz], junk[:, 0:128],
                                     junk[:, 0:sz], start=True, stop=True)

            emit_filler(5, 512)

            # ---- phase A: conv (1x1) -> W^T [hw=256 on 2 chunks, 1028 free] ----
            # Slice (0,512) first: it alone feeds layer-0 weights (rows
            # 0..257), so phase B j=0 + the main loop's first superpair can
            # start while the remaining conv columns are still computing.
            for off, sz in ((0, 512),):
                for m in range(2):
                    ps = psm.tile([128, 2, TP], fp, tag="psmm", name="psA0")
                    psf = ps.rearrange("p a b -> p (a b)")
                    for q in range(8):
                        nc.tensor.matmul(
                            psf[:, :sz], xb[:, q, 128 * m:128 * (m + 1)],
                            cwT[:, q, off:off + sz],
                            start=(q == 0), stop=False)
                    nc.tensor.matmul(
                        psf[:, :sz], ones[:, 0:128], cb[:, off:off + sz],
                        start=False, stop=True)
                    nc.vector.tensor_copy(Wt[:, m, off:off + sz], psf[:, :sz])

            def emit_phaseB(j):
                r0 = 257 * j
                for m in range(2):
                    ps = psm.tile([128, 2, TP], fp, tag="psmm",
                                  name=f"psB{j}{m}")
                    psf = ps.rearrange("p a b -> p (a b)")[:, :NF]
                    for k in range(2):
                        nc.tensor.matmul(
                            psf[:], Wt[:, k, r0 + 128 * m:r0 + 128 * (m + 1)],
                            wfT[:, k, :], start=(k == 0), stop=False)
                    nc.tensor.matmul(psf[:], ones[:, 0:128], wfb[:],
                                     start=False, stop=True)
                    nc.vector.tensor_scalar(wj8[j][:, m, :], psf[:],
                                            AW, None, MULT)
                for c in range(2):
                    psb = psm.tile([128, 2, TP], fp, tag="psmm",
                                   name=f"psBb{j}{c}")
                    psbf = psb.rearrange("p a b -> p (a b)")[:, :1]
                    for k in range(2):
                        nc.tensor.matmul(
                            psbf[:], wfT[:, k, 128 * c:128 * (c + 1)],
                            Wt[:, k, r0 + 256:r0 + 257],
                            start=(k == 0), stop=False)
                    nc.tensor.matmul(psbf[:], wfb[:, 128 * c:128 * (c + 1)],
                                     ones[:, 0:1], start=False, stop=True)
                    nc.vector.tensor_scalar(bA[j][:, c:c + 1], psbf[:],
                                            G[j], None, MULT)
                    if j in bD:
                        nc.vector.tensor_scalar(bD[j][:, c:c + 1], psbf[:],
                                                AW * GPREV[j], None, MULT)

            # phase B j=0 right after the hoisted conv slice so the first
            # superpair's layer 0 can start early
            emit_phaseB(0)

            for m in range(2):
                for off, sz in ((512, 512), (1024, 4)):
                    ps = psm.tile([128, 2, TP], fp, tag="psmm", name="psA")
                    psf = ps.rearrange("p a b -> p (a b)")
                    for q in range(8):
                        nc.tensor.matmul(
                            psf[:, :sz], xb[:, q, 128 * m:128 * (m + 1)],
                            cwT[:, q, off:off + sz],
                            start=(q == 0), stop=False)
                    nc.tensor.matmul(
                        psf[:, :sz], ones[:, 0:128], cb[:, off:off + sz],
                        start=False, stop=True)
                    nc.vector.tensor_copy(Wt[:, m, off:off + sz], psf[:, :sz])

            for j in [1, 2, 3]:
                emit_phaseB(j)

            # ---- main loop: superpairs of 2048 px, DoubleRow fp8 layers ----
            def emit_layer(sp, j, prev, Ty_sb):
                odt = bf if j == L - 1 else f8d
                actj = [ap.tile([128, 2, 2 * TP], odt, tag=f"act{j}{h}",
                                name=f"act{j}{h}_{sp}") for h in range(2)]
                for c in range(2):
                    ps = [psm.tile([128, 2, TP], fp, tag="psmm",
                                   name=f"ps{j}{c}{h}_{sp}") for h in range(2)]
                    for h in range(2):
                        for s_ in range(2):
                            if j == 0:
                                rhs = Ty_sb[:, 0:2,
                                            1024 * h + TP * s_:
                                            1024 * h + TP * (s_ + 1)]
                            else:
                                rhs = prev[h][:, 0:2, TP * s_:TP * (s_ + 1)]
                            nc.tensor.matmul(
                                ps[h][:, s_, :],
                                wj8[j][:, 0:2, 128 * c:128 * (c + 1)], rhs,
                                start=True, stop=True, perf_mode=DR)
                    emit_filler()
                    for h in range(2):
                        psf = ps[h].rearrange("p a b -> p (a b)")
                        path = PATH.get((j, c, h), 'A') if dve_ok else 'A'
                        if path == 'A':
                            nc.scalar.activation(
                                actj[h][:, c, :], psf, PRELU,
                                bias=bA[j][:, c:c + 1], scale=U[j],
                                alpha=alpha)
                        else:
                            t = dp.tile([128, 2 * TP], bf, tag="dvet",
                                        name=f"t{j}{c}{h}_{sp}")
                            nc.vector.tensor_scalar(
                                t[:], psf, bD[j][:, c:c + 1],
                                alpha * U[j], ADD, MULT)
                            nc.vector.scalar_tensor_tensor(
                                actj[h][:, c, :], t[:], 1.0 / alpha, t[:],
                                MULT, MAX)
                return actj

            def emit_last(sp, act3):
                accL = psl.tile([128, TP], fp, tag="pslastb", name=f"accL{sp}")
                for k in range(2):
                    for q in range(4):
                        h, s_ = q // 2, q % 2
                        nc.tensor.matmul(
                            accL[32 * q:32 * q + C2, :], lwT[:, k, :],
                            act3[h][:, k, TP * s_:TP * (s_ + 1)],
                            start=(k == 0), stop=(k == 1),
                            tile_position=(0, 32 * q))
                souf = op.tile([128, TP], fp, tag="souf", name=f"souf{sp}")
                nc.scalar.activation(souf[0:99, :], accL[0:99, :],
                                     SILU, bias=lbrep[0:99, 0:1],
                                     scale=1.0 / G[L - 1])
                for c in range(C2):
                    nc.sync.dma_start(out_r[c, 4 * sp:4 * sp + 4, :],
                                      souf[c:c + 97:32, :])

            # 4-deep layer-skewed software pipeline: at step s emit
            # L3(s-3), L2(s-2), L1(s-1), L0(s) - oldest stream first so the
            # critical path gets scheduler priority, newest fills gaps.
            acts = {}
            for step in range(NSP + 3):
                for j in (3, 2, 1, 0):
                    sp = step - j
                    if not (0 <= sp < NSP):
                        continue
                    if j == 0:
                        Ty_sb = ap.tile([128, 2, 4 * TP], f8d, tag="tysb",
                                        name=f"ty{sp}")
                        nc.sync.dma_start(Ty_sb[:], Ty8_d[:, sp])
                        acts[(sp, 0)] = emit_layer(sp, 0, None, Ty_sb)
                    else:
                        acts[(sp, j)] = emit_layer(
                            sp, j, acts.pop((sp, j - 1)), None)
                    if j == 3:
                        emit_last(sp, acts.pop((sp, 3)))
                        emit_filler()
                if 1 <= step <= 3:
                    # keep HAM warm through pipeline-fill stalls
                    emit_filler(2, 512)
    return nc


def kernel(x, conv_w, conv_b, wfine_w, wfine_b, last1_w, last1_b, prelu_a,
           **_ignored):
    global _last_results
    from concourse.bass_utils import run_bass_kernel_spmd

    x = np.asarray(x)
    B = x.shape[0]
    assert x.shape == (B, C1, 16, 16) and B == 8, x.shape

    conv_w = np.asarray(conv_w, np.float32)      # [1028, 1024]
    conv_b = np.asarray(conv_b, np.float32)      # [1028]
    wfine_w = np.asarray(wfine_w, np.float32)    # [256, 256]
    wfine_b = np.asarray(wfine_b, np.float32)    # [256]
    last1_w = np.asarray(last1_w, np.float32)    # [3, 256]
    last1_b = np.asarray(last1_b, np.float32)    # [3]
    alpha = float(np.asarray(prelu_a).reshape(-1)[0])

    # host-side shared operands (bf16)
    cwT = np.ascontiguousarray(
        conv_w.T.reshape(8, 128, WD).transpose(1, 0, 2)).astype(bf16)
    cb = conv_b.reshape(1, WD).astype(bf16)
    wfT = np.ascontiguousarray(
        wfine_w.T.reshape(2, 128, NF).transpose(1, 0, 2)).astype(bf16)
    wfb = wfine_b.reshape(1, NF).astype(bf16)
    lwT = np.ascontiguousarray(
        last1_w.T.reshape(2, 128, C2).transpose(1, 0, 2)).astype(bf16)
    lbrep = np.zeros((128, 1), np.float32)
    for g in range(4):
        lbrep[32 * g:32 * g + C2, 0] = last1_b
    # layer-0 DoubleRow rhs tables, pre-scaled by GIN: slot 0 = y-part
    # columns (broadcast along x), slot 1 = x-part columns (tiled 16x)
    T8 = GIN * _host_tables()
    Ty8 = np.empty((128, NSP, 2, 4 * TP), np.float32)
    xs = np.tile(np.arange(IMG), 16)
    for sp in range(NSP):
        ys = 16 * sp + np.repeat(np.arange(16), IMG)
        Ty8[:, sp, 0, :] = T8[:, ys]
        Ty8[:, sp, 1, :] = T8[:, xs]
    Ty8 = Ty8.astype(f8)

    nc = _build_program(alpha)

    in_maps = []
    for b in range(B):
        xb = np.ascontiguousarray(
            x[b].reshape(8, 128, NF).transpose(1, 0, 2)).astype(bf16)
        in_maps.append({"xb": xb, "cwT": cwT, "cb": cb, "wfT": wfT,
                        "wfb": wfb, "lwT": lwT, "lbrep": lbrep,
                        "Ty8": Ty8})

    res = run_bass_kernel_spmd(nc, in_maps, list(range(8)))
    _last_results = res
    out = np.stack([res.results[b]["out"].reshape(C2, IMG, IMG)
                    for b in range(B)])
    return out.astype(np.float32)


# ---------------------------------------------------------------------------
# Embedded walrus workaround (kernel.py must be self-contained): this walrus
# build accepts at most ONE sync wait per instruction; Tile attaches several.
# Split them into preceding single-wait NoOps at the BIR-JSON level, and make
# the TileContext tail drain emit one single-wait drain per logical proc.
# ---------------------------------------------------------------------------
import sys as _sys
import types as _types

_patch_mod = _types.ModuleType("bir_patch_embedded")
_patch_src = r'''
import json

def install():
    import concourse.bass_utils as _bu
    import concourse.bass2jax as _b2j
    import concourse.tile as _tile
    from concourse.vector_clock import ScopedClock, VectorClock

    if getattr(_bu, "_wait_legalizer_installed", False):
        return
    _bu._wait_legalizer_installed = True
    _orig_compile = _bu.compile_bir_kernel

    def _merge_ldweights(m):
        """Re-merge tile-legalize's split Ldweights into self-loading
        Matmults so walrus codegen can apply FWL / ldw dedupe."""
        for fn in m.get("functions", []):
            for bb in fn.get("blocks", []):
                instrs = bb.get("instructions", [])
                out = []
                i = 0
                while i < len(instrs):
                    ins = instrs[i]
                    if ins.get("opcode") == "Ldweights":
                        wap = json.dumps(ins["ins"][0], sort_keys=True)
                        # find the next Matmult on this engine using these
                        # weights (stationary operand = ins[1])
                        tgt = None
                        for k in range(i + 1, min(i + 8, len(instrs))):
                            nxt = instrs[k]
                            if nxt.get("engine") != ins.get("engine"):
                                continue
                            if nxt.get("opcode") == "Matmult" and json.dumps(
                                    nxt["ins"][1], sort_keys=True) == wap:
                                tgt = nxt
                            break
                        if tgt is not None:
                            tgt["ldweights"] = True
                            si, ti = ins.get("sync_info") or {}, tgt.setdefault(
                                "sync_info", {"on_wait": [], "on_update": []})
                            ti.setdefault("on_wait", []).extend(
                                si.get("on_wait") or [])
                            ti.setdefault("on_update", []).extend(
                                si.get("on_update") or [])
                            i += 1
                            continue
                    out.append(ins)
                    i += 1
                bb["instructions"] = out
        return m

    def _legalize_waits(bir_json):
        m = json.loads(bir_json)
        m = _merge_ldweights(m)
        cnt = 0
        changed = True
        for fn in m.get("functions", []):
            for bb in fn.get("blocks", []):
                new_instrs = []
                for ins in bb.get("instructions", []):
                    si = ins.get("sync_info")
                    ow = (si or {}).get("on_wait") or []
                    if len(ow) > 1:
                        changed = True
                        for w in ow[:-1]:
                            cnt += 1
                            new_instrs.append({
                                "engine": ins["engine"],
                                "ins": [], "outs": [],
                                "name": "WSPLIT-%d" % cnt,
                                "opcode": "NoOp",
                                "sync_info": {"on_update": [], "on_wait": [w]},
                                "debug": ins.get("debug", 0),
                            })
                        si["on_wait"] = [ow[-1]]
                    new_instrs.append(ins)
                bb["instructions"] = new_instrs
        if not changed:
            return bir_json
        return json.dumps(m).encode()

    def _compile_legalized(bir_json, tmpdir, neff_name="file.neff"):
        return _orig_compile(_legalize_waits(bir_json), tmpdir, neff_name)

    _bu.compile_bir_kernel = _compile_legalized
    _b2j.compile_bir_kernel = _compile_legalized

    import os
    if os.environ.get("BASS_LDW_OPT", "1") != "0":
        _orig_verify = _bu.bir_verify_and_optimise

        def _verify_ldwopt(tmpdir, inp="bir.json", outp="file.neff", arch=None,
                           *, dve_root=None):
            saved = _bu.run_command

            def run_cmd(cmd, **kw):
                cmd = [c.replace("--enable-ldw-opt=false",
                                 "--enable-ldw-opt=true")
                       if isinstance(c, str) else c for c in cmd]
                return saved(cmd, **kw)
            _bu.run_command = run_cmd
            try:
                return _orig_verify(tmpdir, inp, outp, arch, dve_root=dve_root)
            finally:
                _bu.run_command = saved
        _bu.bir_verify_and_optimise = _verify_ldwopt

    def _drain_and_barrier_split(self, tick_clock, wait_clock):
        nc = self.nc
        vclock = tick_clock.global_clock
        n = len(vclock)
        for p in range(n):
            t = vclock[p]
            if t <= 0:
                continue
            v = VectorClock([0] * n)
            v.require_at_least(p, t)
            d = nc.sync.drain()
            wait_clock.add_sem_waits(d.ins, ScopedClock({None: v}))
        nc.all_engine_barrier()
        popped = nc._tile_sem_poison_stack.pop()
        assert popped is self._sem_poison
        nc.clear_and_free_semaphores(list(self.sems.allocated().values()))
        nc.all_engine_barrier()
    _tile.TileContext._drain_and_barrier = _drain_and_barrier_split
'''
exec(_patch_src, _patch_mod.__dict__)
_sys.modules["bir_patch_embedded"] = _patch_mod


# revision 29
# speedup vs baseline: 1.1522x; 1.0653x over previous
"""Trainium2 Bass kernel for nn_CC_Decoder (hypernetwork-decoded per-pixel MLP).

Strategy (8 NeuronCores, data-parallel over batch: one sample per core):

Reference computation per sample:
  W_raw = conv1x1(x)                         # [1028, 256] channel matmul
  Wf    = W_raw @ wfine^T + wfine_b          # [1028, 256]
  layer j weights wj = Wf[257j : 257j+256], bias bj = Wf[257j+256]
  out = PE(coords)  -> 4 x (out @ wj + bj -> PReLU) -> last1 -> SiLU

v2: the four generated-layer GEMMs run as fp8e4 DoubleRow matmuls: weights
and activations are stored as [128 part, 2 k-chunk, free] fp8 tiles, so one
matmul contracts all K=256 at 2x bf16 throughput. Power-of-2 scales keep
fp8 operands in range (PReLU commutes with positive scales; the cumulative
scale is undone by the SiLU activation's scale port). The weight-generation
phases (conv, wfine), the biases, and the last1 layer stay bf16/fp32: the
network output is bias-dominated, so fp8 noise on the matmul paths is
strongly attenuated while the bias path keeps full precision.

The positional-encoding input x2 is an outer sum over (y, x):
x2[(y,x), :] = [u(y)(128) | v(x)(128)] with u = v = T columns. Layer 0's
DoubleRow rhs is a host fp8 table [128, 2, 2048] per superpair: slot 0 the
y-columns (broadcast along x), slot 1 the x-columns (tiled) - the 16 MB x2
tensor never materializes.

Per-layer PReLU+bias is split statically across three engines: ACT does
prelu(scale*psum + bias) in one op; the rest run as a DVE fused op
t = (psum + b')*(alpha*u) followed by max(t/alpha, t) on the otherwise-idle
GPSIMD (or DVE) via scalar_tensor_tensor. The last1 (256->3) matmuls stack
4 pixel-tiles into one PSUM bank at 32-aligned partition offsets via
tile_position col-groups, amortizing SiLU to one instruction per 2048 px.
"""
import numpy as np
import ml_dtypes

bf16 = ml_dtypes.bfloat16
f8 = ml_dtypes.float8_e4m3fn

IMG = 128
NPX = IMG * IMG          # 16384 pixels
NF = 256                 # feature width
C1 = 1024                # conv in-channels
WD = 1028                # conv out-channels (= 4*257)
L = 4                    # generated layers
C2 = 3                   # output channels
TP = 512                 # pixel tile
NT = NPX // TP           # 32 tiles
NSP = NT // 4            # 8 superpairs (2048 px each)
M_ = 64
SIGMA = 10.0

# power-of-2 fp8 scale plan: stored_act_j = G[j] * act_j, stored_w_j = AW * w_j,
# stored layer-0 input tables = GIN * x2. U[j] = G[j]/(AW*Gprev[j]) is the
# psum prescale that recovers G[j]*(w@act + b) before PReLU.
GIN = 8.0
AW = 4096.0
G = [1024.0, 8192.0, 8192.0, 8192.0]
GPREV = [GIN, G[0], G[1], G[2]]
U = [G[j] / (AW * GPREV[j]) for j in range(L)]

# static engine assignment for the per-(layer, chunk, h) PReLU ops:
# 'A' = one ACT op (prelu with bias+scale ports, reads PSUM directly);
# 'D' = DVE pair: fused tensor_scalar t=(psum+b')*(alpha*u), then
# scalar_tensor_tensor out=max(t/alpha, t). The DVE pairs go on layer 3
# because its output is bf16 (for last1): a 2-byte output keeps the DVE
# 2x mode on step 2 (~650 ns vs ~1215 with an fp8 output). GPSIMD can't
# help (no PSUM port; this walrus rejects 2-input ops on Pool) and custom
# DVE ops fail walrus codegen entirely.
PATH = {
    (3, 0, 0): 'D', (3, 0, 1): 'D', (1, 0, 0): 'D', (1, 0, 1): 'D',
    (0, 0, 0): 'D',
}

_last_results = None     # stash for test.py introspection


def _host_tables():
    v0, v1 = -0.99999, 1.0
    r = (v1 - v0) / (2 * IMG)
    seq = v0 + r + 2 * r * np.arange(IMG, dtype=np.float64)
    j = np.arange(M_, dtype=np.float64)
    coeffs = 2.0 * np.pi * (SIGMA ** (j / M_))
    vp = coeffs[:, None] * seq[None, :]          # [64, 128]
    T = np.concatenate([np.cos(vp), np.sin(vp)], axis=0)  # [128, 128]
    return T.astype(np.float32)


def _build_program(alpha: float):
    import concourse.bass as bass
    import concourse.mybir as mybir
    import concourse.tile as tile
    import bir_patch_embedded  # installed below via sys.modules
    bir_patch_embedded.install()

    fp = mybir.dt.float32
    bf = mybir.dt.bfloat16
    f8d = mybir.dt.float8e4
    PRELU = mybir.ActivationFunctionType.Prelu
    SILU = mybir.ActivationFunctionType.Silu
    ADD = mybir.AluOpType.add
    MULT = mybir.AluOpType.mult
    MAX = mybir.AluOpType.max
    DR = mybir.MatmulPerfMode.DoubleRow

    # DVE prelu path needs 0 < alpha <= 1 (max identity + finite 1/alpha)
    dve_ok = 1e-3 <= alpha <= 1.0
    bD_layers = sorted({j for (j, _, _) in PATH}) if dve_ok else []

    nc = bass.Bass()
    xb_d = nc.declare_dram_parameter("xb", [128, 8, NF], bf, isOutput=False)
    cwT_d = nc.declare_dram_parameter("cwT", [128, 8, WD], bf, isOutput=False)
    cb_d = nc.declare_dram_parameter("cb", [1, WD], bf, isOutput=False)
    wfT_d = nc.declare_dram_parameter("wfT", [128, 2, NF], bf, isOutput=False)
    wfb_d = nc.declare_dram_parameter("wfb", [1, NF], bf, isOutput=False)
    lwT_d = nc.declare_dram_parameter("lwT", [128, 2, C2], bf, isOutput=False)
    lbrep_d = nc.declare_dram_parameter("lbrep", [128, 1], fp, isOutput=False)
    Ty8_d = nc.declare_dram_parameter("Ty8", [128, NSP, 2, 4 * TP], f8d,
                                      isOutput=False)
    out_d = nc.declare_dram_parameter("out", [C2, NPX], fp, isOutput=True)
    out_r = out_d.rearrange("c (t x) -> c t x", x=TP)

    with tile.TileContext(nc) as tc:
        with (
            tc.tile_pool(name="wpool", bufs=1) as wp,
            tc.tile_pool(name="actp", bufs=4) as ap,
            tc.tile_pool(name="dvet", bufs=5) as dp,
            tc.tile_pool(name="outp", bufs=2) as op,
            tc.tile_pool(name="psmain", bufs=4, space="PSUM") as psm,
        ):
            # ---- persistent weights / tables ----
            xb = wp.tile([128, 8, NF], bf)
            cwT = wp.tile([128, 8, WD], bf)
            cb = wp.tile([1, WD], bf)
            wfT = wp.tile([128, 2, NF], bf)
            wfb = wp.tile([1, NF], bf)
            lwT = wp.tile([128, 2, C2], bf)
            lbrep = wp.tile([128, 1], fp)
            ones = wp.tile([1, 128], bf)
            Wt = wp.tile([128, 2, WD], bf)           # conv out, transposed (W^T)
            wj8 = [wp.tile([128, 2, NF], f8d, tag=f"wj{j}", name=f"wj{j}")
                   for j in range(L)]
            bA = [wp.tile([128, 2], fp, tag=f"bA{j}", name=f"bA{j}")
                  for j in range(L)]
            bD = {j: wp.tile([128, 2], fp, tag=f"bD{j}", name=f"bD{j}")
                  for j in bD_layers}

            nc.sync.dma_start(xb[:], xb_d[:])
            for q in range(8):
                nc.sync.dma_start(cwT[:, q, :], cwT_d[:, q, :])
            nc.sync.dma_start(cb[:], cb_d[:])
            nc.sync.dma_start(wfT[:], wfT_d[:])
            nc.sync.dma_start(wfb[:], wfb_d[:])
            nc.sync.dma_start(lwT[:], lwT_d[:])
            nc.sync.dma_start(lbrep[:], lbrep_d[:])
            nc.vector.memset(ones[:], 1.0)

            # HAM warmup: junk matmuls keep the PE busy during the input DMA
            # so phase A starts at full clock (ring slot; head-only use).
            junk = wp.tile([128, 512], bf)
            nc.vector.memset(junk[:], 0.5)
            jps = psm.tile([128, 2, TP], fp, tag="psmm", name="warm")
            jpsf = jps.rearrange("p a b -> p (a b)")
            for i in range(10):
                nc.tensor.matmul(jpsf[:, 0:512], junk[:, 0:128], junk[:],
                                 start=(i == 0), stop=(i == 9))

            # ---- phase A: conv (1x1) -> W^T [hw=256 on 2 chunks, 1028 free] ----
            # Slice (0,512) first: it alone feeds layer-0 weights (rows
            # 0..257), so phase B j=0 + the main loop's first superpair can
            # start while the remaining conv columns are still computing.
            for off, sz in ((0, 512),):
                for m in range(2):
                    ps = psm.tile([128, 2, TP], fp, tag="psmm", name="psA0")
                    psf = ps.rearrange("p a b -> p (a b)")
                    for q in range(8):
                        nc.tensor.matmul(
                            psf[:, :sz], xb[:, q, 128 * m:128 * (m + 1)],
                            cwT[:, q, off:off + sz],
                            start=(q == 0), stop=False)
                    nc.tensor.matmul(
                        psf[:, :sz], ones[:, 0:128], cb[:, off:off + sz],
                        start=False, stop=True)
                    nc.vector.tensor_copy(Wt[:, m, off:off + sz], psf[:, :sz])

            def emit_phaseB(j):
                r0 = 257 * j
                for m in range(2):
                    ps = psm.tile([128, 2, TP], fp, tag="psmm",
                                  name=f"psB{j}{m}")
                    psf = ps.rearrange("p a b -> p (a b)")[:, :NF]
                    for k in range(2):
                        nc.tensor.matmul(
                            psf[:], Wt[:, k, r0 + 128 * m:r0 + 128 * (m + 1)],
                            wfT[:, k, :], start=(k == 0), stop=False)
                    nc.tensor.matmul(psf[:], ones[:, 0:128], wfb[:],
                                     start=False, stop=True)
                    nc.vector.tensor_scalar(wj8[j][:, m, :], psf[:],
                                            AW, None, MULT)
                for c in range(2):
                    psb = psm.tile([128, 2, TP], fp, tag="psmm",
                                   name=f"psBb{j}{c}")
                    psbf = psb.rearrange("p a b -> p (a b)")[:, :1]
                    for k in range(2):
                        nc.tensor.matmul(
                            psbf[:], wfT[:, k, 128 * c:128 * (c + 1)],
                            Wt[:, k, r0 + 256:r0 + 257],
                            start=(k == 0), stop=False)
                    nc.tensor.matmul(psbf[:], wfb[:, 128 * c:128 * (c + 1)],
                                     ones[:, 0:1], start=False, stop=True)
                    nc.vector.tensor_scalar(bA[j][:, c:c + 1], psbf[:],
                                            G[j], None, MULT)
                    if j in bD:
                        nc.vector.tensor_scalar(bD[j][:, c:c + 1], psbf[:],
                                                AW * GPREV[j], None, MULT)

            # phase B j=0 right after the hoisted conv slice so the first
            # superpair's layer 0 can start early
            emit_phaseB(0)

            for m in range(2):
                for off, sz in ((512, 512), (1024, 4)):
                    ps = psm.tile([128, 2, TP], fp, tag="psmm", name="psA")
                    psf = ps.rearrange("p a b -> p (a b)")
                    for q in range(8):
                        nc.tensor.matmul(
                            psf[:, :sz], xb[:, q, 128 * m:128 * (m + 1)],
                            cwT[:, q, off:off + sz],
                            start=(q == 0), stop=False)
                    nc.tensor.matmul(
                        psf[:, :sz], ones[:, 0:128], cb[:, off:off + sz],
                        start=False, stop=True)
                    nc.vector.tensor_copy(Wt[:, m, off:off + sz], psf[:, :sz])

            for j in [1, 2, 3]:
                emit_phaseB(j)

            # ---- main loop: superpairs of 2048 px, DoubleRow fp8 layers ----
            def emit_layer(sp, j, prev, Ty_sb):
                odt = bf if j == L - 1 else f8d
                actj = [ap.tile([128, 2, 2 * TP], odt, tag=f"act{j}{h}",
                                name=f"act{j}{h}_{sp}") for h in range(2)]
                # DVE step2 (scalar_tensor_tensor) ops batch AFTER all the
                # psum-reading step1 ops so ring slots free as early as
                # possible (STT reads only the SBUF intermediate).
                stt_q = []
                for c in range(2):
                    ps = [psm.tile([128, 2, TP], fp, tag="psmm",
                                   name=f"ps{j}{c}{h}_{sp}") for h in range(2)]
                    for h in range(2):
                        for s_ in range(2):
                            if j == 0:
                                rhs = Ty_sb[:, 0:2,
                                            1024 * h + TP * s_:
                                            1024 * h + TP * (s_ + 1)]
                            else:
                                rhs = prev[h][:, 0:2, TP * s_:TP * (s_ + 1)]
                            nc.tensor.matmul(
                                ps[h][:, s_, :],
                                wj8[j][:, 0:2, 128 * c:128 * (c + 1)], rhs,
                                start=True, stop=True, perf_mode=DR)
                    for h in range(2):
                        psf = ps[h].rearrange("p a b -> p (a b)")
                        path = PATH.get((j, c, h), 'A') if dve_ok else 'A'
                        if path == 'A':
                            nc.scalar.activation(
                                actj[h][:, c, :], psf, PRELU,
                                bias=bA[j][:, c:c + 1], scale=U[j],
                                alpha=alpha)
                        else:
                            t = dp.tile([128, 2 * TP], bf, tag="dvet",
                                        name=f"t{j}{c}{h}_{sp}")
                            nc.vector.tensor_scalar(
                                t[:], psf, bD[j][:, c:c + 1],
                                alpha * U[j], ADD, MULT)
                            stt_q.append((actj[h][:, c, :], t))
                for out_ap, t in stt_q:
                    nc.vector.scalar_tensor_tensor(
                        out_ap, t[:], 1.0 / alpha, t[:], MULT, MAX)
                return actj

            def emit_last(sp, act3):
                accLt = psm.tile([128, 2, TP], fp, tag="psmm",
                                 name=f"accL{sp}")
                accL = accLt[:, 0, :]
                for k in range(2):
                    for q in range(4):
                        h, s_ = q // 2, q % 2
                        nc.tensor.matmul(
                            accL[32 * q:32 * q + C2, :], lwT[:, k, :],
                            act3[h][:, k, TP * s_:TP * (s_ + 1)],
                            start=(k == 0), stop=(k == 1),
                            tile_position=(0, 32 * q))
                souf = op.tile([128, TP], fp, tag="souf", name=f"souf{sp}")
                nc.scalar.activation(souf[0:99, :], accL[0:99, :],
                                     SILU, bias=lbrep[0:99, 0:1],
                                     scale=1.0 / G[L - 1])
                for c in range(C2):
                    nc.sync.dma_start(out_r[c, 4 * sp:4 * sp + 4, :],
                                      souf[c:c + 97:32, :])

            # 4-deep layer-skewed software pipeline: at step s emit
            # L3(s-3), L2(s-2), L1(s-1), L0(s) - oldest stream first so the
            # critical path gets scheduler priority, newest fills gaps.
            acts = {}
            for step in range(NSP + 3):
                for j in (3, 2, 1, 0):
                    sp = step - j
                    if not (0 <= sp < NSP):
                        continue
                    if j == 0:
                        Ty_sb = ap.tile([128, 2, 4 * TP], f8d, tag="tysb",
                                        name=f"ty{sp}")
                        nc.sync.dma_start(Ty_sb[:], Ty8_d[:, sp])
                        acts[(sp, 0)] = emit_layer(sp, 0, None, Ty_sb)
                    else:
                        acts[(sp, j)] = emit_layer(
                            sp, j, acts.pop((sp, j - 1)), None)
                    if j == 3:
                        emit_last(sp, acts.pop((sp, 3)))

    return nc


def kernel(x, conv_w, conv_b, wfine_w, wfine_b, last1_w, last1_b, prelu_a,
           **_ignored):
    global _last_results
    from concourse.bass_utils import run_bass_kernel_spmd

    x = np.asarray(x)
    B = x.shape[0]
    assert x.shape == (B, C1, 16, 16) and B == 8, x.shape

    conv_w = np.asarray(conv_w, np.float32)      # [1028, 1024]
    conv_b = np.asarray(conv_b, np.float32)      # [1028]
    wfine_w = np.asarray(wfine_w, np.float32)    # [256, 256]
    wfine_b = np.asarray(wfine_b, np.float32)    # [256]
    last1_w = np.asarray(last1_w, np.float32)    # [3, 256]
    last1_b = np.asarray(last1_b, np.float32)    # [3]
    alpha = float(np.asarray(prelu_a).reshape(-1)[0])

    # host-side shared operands (bf16)
    cwT = np.ascontiguousarray(
        conv_w.T.reshape(8, 128, WD).transpose(1, 0, 2)).astype(bf16)
    cb = conv_b.reshape(1, WD).astype(bf16)
    wfT = np.ascontiguousarray(
        wfine_w.T.reshape(2, 128, NF).transpose(1, 0, 2)).astype(bf16)
    wfb = wfine_b.reshape(1, NF).astype(bf16)
    lwT = np.ascontiguousarray(
        last1_w.T.reshape(2, 128, C2).transpose(1, 0, 2)).astype(bf16)
    lbrep = np.zeros((128, 1), np.float32)
    for g in range(4):
        lbrep[32 * g:32 * g + C2, 0] = last1_b
    # layer-0 DoubleRow rhs tables, pre-scaled by GIN: slot 0 = y-part
    # columns (broadcast along x), slot 1 = x-part columns (tiled 16x)
    T8 = GIN * _host_tables()
    Ty8 = np.empty((128, NSP, 2, 4 * TP), np.float32)
    xs = np.tile(np.arange(IMG), 16)
    for sp in range(NSP):
        ys = 16 * sp + np.repeat(np.arange(16), IMG)
        Ty8[:, sp, 0, :] = T8[:, ys]
        Ty8[:, sp, 1, :] = T8[:, xs]
    Ty8 = Ty8.astype(f8)

    nc = _build_program(alpha)

    in_maps = []
    for b in range(B):
        xb = np.ascontiguousarray(
            x[b].reshape(8, 128, NF).transpose(1, 0, 2)).astype(bf16)
        in_maps.append({"xb": xb, "cwT": cwT, "cb": cb, "wfT": wfT,
                        "wfb": wfb, "lwT": lwT, "lbrep": lbrep,
                        "Ty8": Ty8})

    res = run_bass_kernel_spmd(nc, in_maps, list(range(8)))
    _last_results = res
    out = np.stack([res.results[b]["out"].reshape(C2, IMG, IMG)
                    for b in range(B)])
    return out.astype(np.float32)


# ---------------------------------------------------------------------------
# Embedded walrus workaround (kernel.py must be self-contained): this walrus
# build accepts at most ONE sync wait per instruction; Tile attaches several.
# Split them into preceding single-wait NoOps at the BIR-JSON level, and make
# the TileContext tail drain emit one single-wait drain per logical proc.
# ---------------------------------------------------------------------------
import sys as _sys
import types as _types

_patch_mod = _types.ModuleType("bir_patch_embedded")
_patch_src = r'''
import json

def install():
    import concourse.bass_utils as _bu
    import concourse.bass2jax as _b2j
    import concourse.tile as _tile
    from concourse.vector_clock import ScopedClock, VectorClock

    if getattr(_bu, "_wait_legalizer_installed", False):
        return
    _bu._wait_legalizer_installed = True
    _orig_compile = _bu.compile_bir_kernel

    def _merge_ldweights(m):
        """Re-merge tile-legalize's split Ldweights into self-loading
        Matmults so walrus codegen can apply FWL / ldw dedupe."""
        for fn in m.get("functions", []):
            for bb in fn.get("blocks", []):
                instrs = bb.get("instructions", [])
                out = []
                i = 0
                while i < len(instrs):
                    ins = instrs[i]
                    if ins.get("opcode") == "Ldweights":
                        wap = json.dumps(ins["ins"][0], sort_keys=True)
                        # find the next Matmult on this engine using these
                        # weights (stationary operand = ins[1])
                        tgt = None
                        for k in range(i + 1, min(i + 8, len(instrs))):
                            nxt = instrs[k]
                            if nxt.get("engine") != ins.get("engine"):
                                continue
                            if nxt.get("opcode") == "Matmult" and json.dumps(
                                    nxt["ins"][1], sort_keys=True) == wap:
                                tgt = nxt
                            break
                        if tgt is not None:
                            tgt["ldweights"] = True
                            si, ti = ins.get("sync_info") or {}, tgt.setdefault(
                                "sync_info", {"on_wait": [], "on_update": []})
                            ti.setdefault("on_wait", []).extend(
                                si.get("on_wait") or [])
                            ti.setdefault("on_update", []).extend(
                                si.get("on_update") or [])
                            i += 1
                            continue
                    out.append(ins)
                    i += 1
                bb["instructions"] = out
        return m

    def _legalize_waits(bir_json):
        m = json.loads(bir_json)
        m = _merge_ldweights(m)
        cnt = 0
        changed = True
        for fn in m.get("functions", []):
            for bb in fn.get("blocks", []):
                new_instrs = []
                for ins in bb.get("instructions", []):
                    si = ins.get("sync_info")
                    ow = (si or {}).get("on_wait") or []
                    if len(ow) > 1:
                        changed = True
                        for w in ow[:-1]:
                            cnt += 1
                            new_instrs.append({
                                "engine": ins["engine"],
                                "ins": [], "outs": [],
                                "name": "WSPLIT-%d" % cnt,
                                "opcode": "NoOp",
                                "sync_info": {"on_update": [], "on_wait": [w]},
                                "debug": ins.get("debug", 0),
                            })
                        si["on_wait"] = [ow[-1]]
                    new_instrs.append(ins)
                bb["instructions"] = new_instrs
        if not changed:
            return bir_json
        return json.dumps(m).encode()

    def _compile_legalized(bir_json, tmpdir, neff_name="file.neff"):
        return _orig_compile(_legalize_waits(bir_json), tmpdir, neff_name)

    _bu.compile_bir_kernel = _compile_legalized
    _b2j.compile_bir_kernel = _compile_legalized

    import os
    if os.environ.get("BASS_LDW_OPT", "1") != "0":
        _orig_verify = _bu.bir_verify_and_optimise

        def _verify_ldwopt(tmpdir, inp="bir.json", outp="file.neff", arch=None,
                           *, dve_root=None):
            saved = _bu.run_command

            def run_cmd(cmd, **kw):
                cmd = [c.replace("--enable-ldw-opt=false",
                                 "--enable-ldw-opt=true")
                       if isinstance(c, str) else c for c in cmd]
                return saved(cmd, **kw)
            _bu.run_command = run_cmd
            try:
                return _orig_verify(tmpdir, inp, outp, arch, dve_root=dve_root)
            finally:
                _bu.run_command = saved
        _bu.bir_verify_and_optimise = _verify_ldwopt

    def _drain_and_barrier_split(self, tick_clock, wait_clock):
        nc = self.nc
        vclock = tick_clock.global_clock
        n = len(vclock)
        for p in range(n):
            t = vclock[p]
            if t <= 0:
                continue
            v = VectorClock([0] * n)
            v.require_at_least(p, t)
            d = nc.sync.drain()
            wait_clock.add_sem_waits(d.ins, ScopedClock({None: v}))
        nc.all_engine_barrier()
        popped = nc._tile_sem_poison_stack.pop()
        assert popped is self._sem_poison
        nc.clear_and_free_semaphores(list(self.sems.allocated().values()))
        nc.all_engine_barrier()
    _tile.TileContext._drain_and_barrier = _drain_and_barrier_split
'''
exec(_patch_src, _patch_mod.__dict__)
_sys.modules["bir_patch_embedded"] = _patch_mod
